# revision 28
# baseline (speedup 1.0000x reference)
"""FFM layer (nn_FFM_Layer) Trainium2 Bass kernel.

Reference computation (B=4096, 13 dense fields, 26 sparse fields with vocab
1000 each, FIELD_NUM=39, K=16):

    idx        = sparse + offsets                      # [B, 26] global ids
    first      = w0 + dense @ w[:13] + sum_j w[idx]    # [B, 1]
    field_f    = einsum('bd,dfk', dense, v[:13]) + sum_j v[idx]   # [B,39,16]
    s          = field_f.sum(1)                        # [B, 16]
    second     = 0.5*(||s||^2 - sum_fk field_f^2)      # [B]
    out        = first + second[:, None]

Strategy (data-parallel over batch, 8 cores x 512 samples, no collectives):
  * Host packs an augmented table V_AUG [26013, 640] f32:
      cols [0:624]  = v.reshape(26013, 39*16)
      col  624      = w[:, 0]   (+ w0 folded into rows of sparse table 0,
                                 which every sample hits exactly once)
      cols [625:640]= 0         (pad rows to 2560 B; dma_gather requires
                                 elem_size_bytes % 256 == 0)
    f32 storage is REQUIRED: outputs can be ~2e-3 while the cancelling
    norm terms are O(40); fp16 table storage injects ~1e-3 abs error and
    blows the 2e-2 max-rel gate (measured 2.2 max rel).
  * Each core dma_gathers its 512*26 rows (SWDGE mlp ucode).  Gathers are
    sample-chunk-major (4-5 calls of 1-7 fields x 128 samples per chunk)
    so each chunk's FM epilogue + output DMA overlap later chunks.
    Trace-measured structure (this is DMA-bytes-bound):
      - per-core DMA bus sustains ~392 GB/s => 34.1 MB of gathers ~87 us
      - 2 SWDGE queues keep call completions IN ORDER; 4 queues make all
        in-flight calls complete simultaneously late (bus dips to ~270)
      - ring capacity = dynamic_dma_scratch_size/16 = 1024 descs/queue
      - single_packet=True is ~10% faster end to end
      - first gather can't start before ~16 us: ~6.3 us framework preamble
        + ~10 us HWDGE pipeline latency of the idx-tile DMA (+ Q7 mlp
        library load, which overlaps)
      - gather-completion sems (+16, one inc per SDMA engine) trail the
        last bytes by ~5-7 us at stream end: engine E79 runs ~10% slower
        than the other 15 (dur_sum 49-51 vs 44-46 us) and every sem waits
        on its backlog; the engine set is runtime-fixed, not controllable
  * Fold pipeline per gather call: DVE folds the call's nf cols into one
    (pairwise adds; float32r identity matmuls were tried for PE offload
    but the BIR verifier requires fp32r-rounded producers = lossy), PE
    accumulates the fold cols into the chunk's PSUM chain with identity
    matmuls; the final col is added by DVE with the single PSUM read.
    Chunk 3's big groups are interleaved into chunk 2's stream (ISSUE)
    and it ends in tiny calls (2,2,1,1 fields) folded as a reassociated
    tree, so the post-DMA DVE backlog is ~5 us instead of ~12.  Tiny
    calls draw from a dedicated gtail tile pool so their desc-gen never
    waits on a big gather tile still being folded.
  * PE also seeds each chunk's PSUM with the dense [13,128]x[13,640]
    matmul (dense^T prepared host-side; col 624 adds dense @ w[:13]).
  * Pad cols 625-639 carry host-precomputed sum_f v[a,f,k] for k=1..15,
    so the fold/PSUM pipeline produces s_1..s_15 for free; the epilogue
    reduces only s_0 on-chip (39 strided elems, 206 ns vs 1.2 us) and
    ACT Square+accum_out supplies both norms.
  * Head: input DMAs split across the two HWDGE queues (sync + scalar),
    idxs as one DMA, num_idxs registers hoisted (one MOVE per distinct
    call size), and a zero-idx warmup gather absorbs the cold-ucode cost
    while the Q7 mlp library load (~16.5 us, the head floor) completes.
"""

import sys

if "/opt/trn_rl_repo" not in sys.path:
    sys.path.insert(0, "/opt/trn_rl_repo")

import numpy as np

import concourse.bacc as bacc
import concourse.bass as bass
import concourse.tile as tile
from concourse import mybir
from concourse.bass_utils import run_bass_kernel_spmd

# Problem constants (hardcoded per harness contract)
B = 4096
N_DENSE = 13
N_SPARSE = 26
FEAT_PER_SPARSE = 1000
FIELD_NUM = 39
FEATURE_NUM = 26013
K = 16
N_CORES = 8
BC = B // N_CORES          # 512 samples per core
ROW = 640                  # padded row: 624 v + 1 w + 15 zeros (2560 B)
VCOLS = FIELD_NUM * K      # 624
P = 128
SCHUNKS = BC // P          # 4 sample chunks of 128 per core
# per-chunk gather calls: field groups (sum 26), each call = nf*128 idxs
FGROUPS = [7, 7, 6, 6]
FGROUPS_FIRST = [2, 4, 4, 4, 6, 6]     # small ramp-up calls keep rings fed
FGROUPS_LAST = [7, 7, 6, 2, 2, 1, 1]   # tiny tail calls
CHUNK_GROUPS = [FGROUPS_FIRST, FGROUPS, FGROUPS, FGROUPS_LAST]
# Issue order interleaves chunks 2/3 so chunk 3's big folds land mid-stream
# and only single/dual-field folds + the epilogues remain after the last byte.
ISSUE = ([(0, g) for g in range(len(FGROUPS_FIRST))]
         + [(1, g) for g in range(len(FGROUPS))]
         + [(2, 0), (2, 1), (2, 2), (3, 0), (3, 1), (3, 2), (2, 3)]
         + [(3, 3), (3, 4), (3, 5), (3, 6)])
IDX_COLS_SC = N_SPARSE * P // 16   # 208 idx cols per sample chunk
N_QUEUES = 2

F32 = mybir.dt.float32
I16 = mybir.dt.int16


def build_program():
    """Build + compile the single-core SPMD bass program."""
    nc = bacc.Bacc("TRN2", target_bir_lowering=False, debug=False,
                   num_swdge_queues=N_QUEUES)

    vaug_t = nc.dram_tensor("vaug", [FEATURE_NUM, ROW], F32, kind="ExternalInput")
    dense_t = nc.dram_tensor("dense_t", [N_DENSE, BC], F32, kind="ExternalInput")
    idxs_t = nc.dram_tensor("idxs", [P, SCHUNKS * IDX_COLS_SC], I16,
                            kind="ExternalInput")
    ident_t = nc.dram_tensor("ident", [P, P], F32, kind="ExternalInput")
    out_t = nc.dram_tensor("out", [P, SCHUNKS], F32, kind="ExternalOutput")

    def acc_mm(ps, rhs_col, start, stop):
        """Accumulate one [128, ROW] column into the psum chain."""
        nc.tensor.matmul(out=ps[:, 0:512], lhsT=ident[:], rhs=rhs_col[:, 0:512],
                         start=start, stop=stop)
        nc.tensor.matmul(out=ps[:, 512:ROW], lhsT=ident[:],
                         rhs=rhs_col[:, 512:ROW], start=start, stop=stop)

    with tile.TileContext(nc) as tc:
        with (
            tc.tile_pool(name="main", bufs=1) as main,
            tc.tile_pool(name="gath", bufs=7) as gath,
            tc.tile_pool(name="gtail", bufs=4) as gtail,
            tc.tile_pool(name="fold", bufs=3) as fold,
            tc.tile_pool(name="small", bufs=2) as small,
            tc.tile_pool(name="psum", bufs=4, space="PSUM") as psum,
        ):
            # idxs in one DMA on the sync HWDGE queue; the other inputs go
            # on the scalar (Activation) HWDGE queue so configs overlap.
            # (The first gather is gated at ~16.5 us by the Q7 mlp library
            # load, not by the idx DMA — warmup DMAs measurably don't help.)
            idx_sb = main.tile([P, SCHUNKS * IDX_COLS_SC], I16, tag="idx")
            nc.sync.dma_start(idx_sb[:], idxs_t[:])
            vaug13 = main.tile([N_DENSE, ROW], F32)
            nc.scalar.dma_start(vaug13[:], vaug_t[0:N_DENSE, :])
            dt_sb = main.tile([N_DENSE, BC], F32)
            nc.scalar.dma_start(dt_sb[:], dense_t[:])
            ident = main.tile([P, P], F32)
            nc.scalar.dma_start(ident[:], ident_t[:])

            # hoist num_idxs registers: one MOVE per distinct call size
            nf_sizes = sorted({nf for fg in (FGROUPS, FGROUPS_FIRST, FGROUPS_LAST)
                               for nf in fg})
            nidx_regs = {nf: nc.gpsimd.to_reg(nf * P) for nf in nf_sizes}

            res = main.tile([P, SCHUNKS], F32)

            # Warmup gather: zeroed idxs (no DMA dependency) gather row 0 into
            # a scratch tile as soon as the Q7 mlp library lands.  Warms the
            # ucode icache so the first real calls desc-gen at ~2.5 ns/row
            # instead of ~10, and primes the SDMA path before the stream.
            zidx = main.tile([P, P // 16], I16, tag="zidx")
            nc.gpsimd.memset(zidx[:], 0)
            gw = gtail.tile([P, 2, ROW], F32, tag="gt")
            nc.gpsimd.dma_gather(
                gw[:, :1, :], vaug_t[:], zidx[:], P, nidx_regs[1], ROW,
                single_packet=True, queue_num=0,
            )

            def fold_call(g, nf, gi):
                """DVE fold: call's nf gathered cols -> one column."""
                if nf == 1:
                    return g[:, 0, :]
                t1 = fold.tile([P, 3, ROW], F32, tag="t1")
                t2 = small.tile([P, ROW], F32, tag=f"t2_{gi % 2}")
                if nf >= 6:
                    nc.vector.tensor_add(t1[:], g[:, 0:3, :], g[:, 3:6, :])
                    nc.vector.tensor_add(t2[:], t1[:, 0, :], t1[:, 1, :])
                    nc.vector.tensor_add(t2[:], t2[:], t1[:, 2, :])
                    if nf == 7:
                        nc.vector.tensor_add(t2[:], t2[:], g[:, 6, :])
                elif nf in (4, 5):
                    nc.vector.tensor_add(t1[:, 0:2, :], g[:, 0:2, :], g[:, 2:4, :])
                    nc.vector.tensor_add(t2[:], t1[:, 0, :], t1[:, 1, :])
                    if nf == 5:
                        nc.vector.tensor_add(t2[:], t2[:], g[:, 4, :])
                else:  # nf == 2
                    nc.vector.tensor_add(t2[:], g[:, 0, :], g[:, 1, :])
                return t2

            def epilogue(c, fld):
                """FM identity epilogue for chunk c given complete fld."""
                blk = fld[:, 0:VCOLS]             # [128, 624] = field_f
                sq = fold.tile([P, VCOLS], F32, tag="sq")
                q = small.tile([P, 1], F32, tag="q")
                nc.scalar.activation(
                    sq[:], blk, mybir.ActivationFunctionType.Square,
                    accum_out=q[:],
                )
                # s_k for k=1..15 rides the gather in pad cols 625-639
                # (host packs sum_f v[a,f,k] there); only s_0 needs an
                # on-chip reduce, over 39 strided elements instead of 624.
                s0 = small.tile([P, 1], F32, tag="s0")
                blk_kf = blk.rearrange("p (f k) -> p k f", k=16)
                nc.vector.tensor_reduce(
                    out=s0[:], in_=blk_kf[:, 0:1, :], op=mybir.AluOpType.add,
                    axis=mybir.AxisListType.X,
                )
                s2 = small.tile([P, 15], F32, tag="s2")
                sn15 = small.tile([P, 1], F32, tag="sn15")
                nc.scalar.activation(
                    s2[:], fld[:, VCOLS + 1 : VCOLS + 16],
                    mybir.ActivationFunctionType.Square,
                    accum_out=sn15[:],
                )
                s0sq = small.tile([P, 1], F32, tag="s0sq")
                nc.vector.tensor_tensor(
                    out=s0sq[:], in0=s0[:], in1=s0[:],
                    op=mybir.AluOpType.mult,
                )
                # diff = (sn15 + s0^2) - q  ==  ||s||^2 - sum field_f^2
                diff = small.tile([P, 1], F32, tag="diff")
                nc.vector.scalar_tensor_tensor(
                    out=diff[:], in0=sn15[:], scalar=s0sq[:], in1=q[:],
                    op0=mybir.AluOpType.add, op1=mybir.AluOpType.subtract,
                )
                # out = 0.5*diff + (w-sum incl. w0 and dense first-order)
                nc.scalar.activation(
                    res[:, c : c + 1],
                    diff[:],
                    mybir.ActivationFunctionType.Identity,
                    bias=fld[:, VCOLS : VCOLS + 1],
                    scale=0.5,
                )
                nc.sync.dma_start(out_t[:, c : c + 1], res[:, c : c + 1])

            ps_c = {}      # chunk -> psum tile
            icol_c = {}    # chunk -> idx column offset consumed
            c3_cols = []   # chunk 3's DVE-chained columns (groups 3+)
            call_no = 1    # warmup gather above was call 0 on queue 0
            for c, gi in ISSUE:
                fgroups = CHUNK_GROUPS[c]
                ngroups = len(fgroups)
                nf = fgroups[gi]
                if gi == 0:
                    # dense part seeds this chunk's PSUM accumulation chain
                    ps = psum.tile([P, ROW], F32, tag="ps")
                    ps_c[c] = ps
                    icol_c[c] = 0
                    lhs_d = dt_sb[:, c * P : (c + 1) * P]
                    nc.tensor.matmul(out=ps[:, 0:512], lhsT=lhs_d,
                                     rhs=vaug13[:, 0:512], start=True, stop=False)
                    nc.tensor.matmul(out=ps[:, 512:ROW], lhsT=lhs_d,
                                     rhs=vaug13[:, 512:ROW], start=True, stop=False)
                ps = ps_c[c]
                icol = icol_c[c]

                n_idx = nf * P
                if nf <= 2:
                    g = gtail.tile([P, 2, ROW], F32, tag="gt")
                else:
                    g = gath.tile([P, 7, ROW], F32, tag="g")
                nc.gpsimd.dma_gather(
                    g[:, :nf, :],
                    vaug_t[:],
                    idx_sb[:, c * IDX_COLS_SC + icol
                           : c * IDX_COLS_SC + icol + n_idx // 16],
                    n_idx,
                    nidx_regs[nf],
                    ROW,
                    single_packet=True,
                    queue_num=call_no % N_QUEUES,
                )
                icol_c[c] = icol + n_idx // 16
                call_no += 1

                t2 = fold_call(g, nf, gi)

                if c < 3:
                    # PE accumulates all but the final group's col; final
                    # col joins via the DVE PSUM read (keeps PE off the
                    # chunk tail).
                    if gi < ngroups - 1:
                        acc_mm(ps, t2, start=False, stop=gi == ngroups - 2)
                    else:
                        fld = fold.tile([P, ROW], F32, tag="fld")
                        nc.vector.tensor_add(fld[:], t2[:], ps[:])
                        epilogue(c, fld)
                else:
                    # chunk 3: only the three big groups go through PE (the
                    # stop fires mid-stream); the tiny tail groups fold on
                    # DVE into fld in ARRIVAL order starting from the PSUM
                    # read, so after the final single-field call lands only
                    # one 640-col add + the epilogue remain.
                    if gi <= 2:
                        acc_mm(ps, t2, start=False, stop=gi == 2)
                    elif gi == 3:
                        fld3 = fold.tile([P, ROW], F32, tag="fld")
                        nc.vector.tensor_add(fld3[:], t2[:], ps[:])
                        c3_cols.append(fld3)
                    else:
                        fld3 = c3_cols[0]
                        nc.vector.tensor_add(fld3[:], fld3[:], t2[:])
                        if gi == ngroups - 1:
                            epilogue(3, fld3)

    nc.compile()
    return nc


def prep_inputs(dense_inputs, sparse_inputs, w0, w, v):
    """Host-side shard/pack: build per-core in_maps."""
    dense = np.asarray(dense_inputs, np.float32)
    sparse = np.asarray(sparse_inputs)
    w0 = np.asarray(w0, np.float32)
    w = np.asarray(w, np.float32)
    v = np.asarray(v, np.float32)

    vaug = np.zeros((FEATURE_NUM, ROW), np.float32)
    vaug[:, :VCOLS] = v.reshape(FEATURE_NUM, VCOLS)
    vaug[:, VCOLS] = w[:, 0]
    # fold w0 into sparse table 0 (each sample hits it exactly once)
    vaug[N_DENSE : N_DENSE + FEAT_PER_SPARSE, VCOLS] += w0[0]
    # pad cols 625-639 carry sum_f v[a,f,k] for k=1..15: the fold pipeline
    # then produces s_k for free and the epilogue only reduces s_0 on-chip
    vaug[:, VCOLS + 1 :] = v.sum(axis=1)[:, 1:16]

    offs = N_DENSE + FEAT_PER_SPARSE * np.arange(N_SPARSE, dtype=np.int64)
    gidx = (sparse.astype(np.int64) + offs[None, :]).astype(np.int16)  # [B, 26]

    in_maps = []
    for core in range(N_CORES):
        sl = slice(core * BC, (core + 1) * BC)
        dt = np.ascontiguousarray(dense[sl].T)          # [13, 512]
        idxc = gidx[sl]                                 # [512, 26]
        buf = np.zeros((P, SCHUNKS * IDX_COLS_SC), np.int16)
        off_c = 0
        for c in range(SCHUNKS):
            rows = idxc[c * P : (c + 1) * P]            # [128, 26]
            fbase = 0
            for nf in (FGROUPS_FIRST if c == 0 else
                       (FGROUPS_LAST if c == SCHUNKS - 1 else FGROUPS)):
                n = nf * P
                # call order: i = f_local*128 + p  ->  row idx[p, fbase+f]
                seg = np.ascontiguousarray(
                    rows[:, fbase : fbase + nf].T
                ).reshape(-1)                           # [nf*128]
                wrapped = seg.reshape(n // 16, 16).T    # [16, n/16]
                buf[:, off_c : off_c + n // 16] = np.tile(wrapped, (8, 1))
                fbase += nf
                off_c += n // 16
        in_maps.append({"vaug": vaug, "dense_t": dt, "idxs": buf,
                        "ident": np.eye(P, dtype=np.float32)})
    return in_maps


_NC_CACHE = None


def kernel(dense_inputs, sparse_inputs, w0, w, v):
    global _NC_CACHE
    if _NC_CACHE is None:
        _NC_CACHE = build_program()
    nc = _NC_CACHE
    in_maps = prep_inputs(dense_inputs, sparse_inputs, w0, w, v)
    res = run_bass_kernel_spmd(nc, in_maps, core_ids=list(range(N_CORES)))
    outs = []
    for r in res.results:
        o = r["out"]                                    # [128, 4]
        outs.append(np.ascontiguousarray(o.T).reshape(BC, 1))
    return np.concatenate(outs, axis=0).astype(np.float32)


# revision 29
# speedup vs baseline: 1.0220x; 1.0220x over previous
"""FFM layer (nn_FFM_Layer) Trainium2 Bass kernel.

Reference computation (B=4096, 13 dense fields, 26 sparse fields with vocab
1000 each, FIELD_NUM=39, K=16):

    idx        = sparse + offsets                      # [B, 26] global ids
    first      = w0 + dense @ w[:13] + sum_j w[idx]    # [B, 1]
    field_f    = einsum('bd,dfk', dense, v[:13]) + sum_j v[idx]   # [B,39,16]
    s          = field_f.sum(1)                        # [B, 16]
    second     = 0.5*(||s||^2 - sum_fk field_f^2)      # [B]
    out        = first + second[:, None]

Strategy (data-parallel over batch, 8 cores x 512 samples, no collectives):
  * Host packs an augmented table V_AUG [26013, 640] f32:
      cols [0:624]  = v.reshape(26013, 39*16)
      col  624      = w[:, 0]   (+ w0 folded into rows of sparse table 0,
                                 which every sample hits exactly once)
      cols [625:640]= 0         (pad rows to 2560 B; dma_gather requires
                                 elem_size_bytes % 256 == 0)
    f32 storage is REQUIRED: outputs can be ~2e-3 while the cancelling
    norm terms are O(40); fp16 table storage injects ~1e-3 abs error and
    blows the 2e-2 max-rel gate (measured 2.2 max rel).
  * Each core dma_gathers its 512*26 rows (SWDGE mlp ucode).  Gathers are
    sample-chunk-major (4-5 calls of 1-7 fields x 128 samples per chunk)
    so each chunk's FM epilogue + output DMA overlap later chunks.
    Trace-measured structure (this is DMA-bytes-bound):
      - per-core DMA bus sustains ~392 GB/s => 34.1 MB of gathers ~87 us
      - 2 SWDGE queues keep call completions IN ORDER; 4 queues make all
        in-flight calls complete simultaneously late (bus dips to ~270)
      - ring capacity = dynamic_dma_scratch_size/16 = 1024 descs/queue
      - single_packet=False is ~10% faster end to end
      - first gather can't start before ~16 us: ~6.3 us framework preamble
        + ~10 us HWDGE pipeline latency of the idx-tile DMA (+ Q7 mlp
        library load, which overlaps)
      - gather-completion sems (+16, one inc per SDMA engine) trail the
        last bytes by ~5-7 us at stream end: engine E79 runs ~10% slower
        than the other 15 (dur_sum 49-51 vs 44-46 us) and every sem waits
        on its backlog; the engine set is runtime-fixed, not controllable
  * Fold pipeline per gather call: DVE folds the call's nf cols into one
    (pairwise adds; float32r identity matmuls were tried for PE offload
    but the BIR verifier requires fp32r-rounded producers = lossy), PE
    accumulates the fold cols into the chunk's PSUM chain with identity
    matmuls; the final col is added by DVE with the single PSUM read.
    Chunk 3's big groups are interleaved into chunk 2's stream (ISSUE)
    and it ends in tiny calls (2,2,1,1 fields) folded as a reassociated
    tree, so the post-DMA DVE backlog is ~5 us instead of ~12.  Tiny
    calls draw from a dedicated gtail tile pool so their desc-gen never
    waits on a big gather tile still being folded.
  * PE also seeds each chunk's PSUM with the dense [13,128]x[13,640]
    matmul (dense^T prepared host-side; col 624 adds dense @ w[:13]).
  * Pad cols 625-639 carry host-precomputed sum_f v[a,f,k] for k=1..15,
    so the fold/PSUM pipeline produces s_1..s_15 for free; the epilogue
    reduces only s_0 on-chip (39 strided elems, 206 ns vs 1.2 us) and
    ACT Square+accum_out supplies both norms.
  * Head: input DMAs split across the two HWDGE queues (sync + scalar),
    idxs as one DMA, num_idxs registers hoisted (one MOVE per distinct
    call size), and a zero-idx warmup gather absorbs the cold-ucode cost
    while the Q7 mlp library load (~16.5 us, the head floor) completes.
"""

import sys

if "/opt/trn_rl_repo" not in sys.path:
    sys.path.insert(0, "/opt/trn_rl_repo")

import numpy as np

import concourse.bacc as bacc
import concourse.bass as bass
import concourse.tile as tile
from concourse import mybir
from concourse.bass_utils import run_bass_kernel_spmd

# Problem constants (hardcoded per harness contract)
B = 4096
N_DENSE = 13
N_SPARSE = 26
FEAT_PER_SPARSE = 1000
FIELD_NUM = 39
FEATURE_NUM = 26013
K = 16
N_CORES = 8
BC = B // N_CORES          # 512 samples per core
ROW = 640                  # padded row: 624 v + 1 w + 15 zeros (2560 B)
VCOLS = FIELD_NUM * K      # 624
P = 128
SCHUNKS = BC // P          # 4 sample chunks of 128 per core
# per-chunk gather calls: field groups (sum 26), each call = nf*128 idxs
FGROUPS = [7, 7, 6, 6]
FGROUPS_FIRST = [2, 4, 4, 4, 6, 6]     # small ramp-up calls keep rings fed
FGROUPS_LAST = [7, 7, 6, 2, 2, 1, 1]   # tiny tail calls
CHUNK_GROUPS = [FGROUPS_FIRST, FGROUPS, FGROUPS, FGROUPS_LAST]
# Issue order interleaves chunks 2/3 so chunk 3's big folds land mid-stream
# and only single/dual-field folds + the epilogues remain after the last byte.
ISSUE = ([(0, g) for g in range(len(FGROUPS_FIRST))]
         + [(1, g) for g in range(len(FGROUPS))]
         + [(2, 0), (2, 1), (2, 2), (3, 0), (3, 1), (3, 2), (2, 3)]
         + [(3, 3), (3, 4), (3, 5), (3, 6)])
IDX_COLS_SC = N_SPARSE * P // 16   # 208 idx cols per sample chunk
N_QUEUES = 2

F32 = mybir.dt.float32
I16 = mybir.dt.int16


def build_program():
    """Build + compile the single-core SPMD bass program."""
    nc = bacc.Bacc("TRN2", target_bir_lowering=False, debug=False,
                   num_swdge_queues=N_QUEUES)

    vaug_t = nc.dram_tensor("vaug", [FEATURE_NUM, ROW], F32, kind="ExternalInput")
    dense_t = nc.dram_tensor("dense_t", [N_DENSE, BC], F32, kind="ExternalInput")
    idxs_t = nc.dram_tensor("idxs", [P, SCHUNKS * IDX_COLS_SC], I16,
                            kind="ExternalInput")
    ident_t = nc.dram_tensor("ident", [P, P], F32, kind="ExternalInput")
    out_t = nc.dram_tensor("out", [P, SCHUNKS], F32, kind="ExternalOutput")

    def acc_mm(ps, rhs_col, start, stop):
        """Accumulate one [128, ROW] column into the psum chain."""
        nc.tensor.matmul(out=ps[:, 0:512], lhsT=ident[:], rhs=rhs_col[:, 0:512],
                         start=start, stop=stop)
        nc.tensor.matmul(out=ps[:, 512:ROW], lhsT=ident[:],
                         rhs=rhs_col[:, 512:ROW], start=start, stop=stop)

    with tile.TileContext(nc) as tc:
        with (
            tc.tile_pool(name="main", bufs=1) as main,
            tc.tile_pool(name="gath", bufs=7) as gath,
            tc.tile_pool(name="gtail", bufs=4) as gtail,
            tc.tile_pool(name="fold", bufs=3) as fold,
            tc.tile_pool(name="small", bufs=2) as small,
            tc.tile_pool(name="psum", bufs=4, space="PSUM") as psum,
        ):
            # idxs in one DMA on the sync HWDGE queue; the other inputs go
            # on the scalar (Activation) HWDGE queue so configs overlap.
            # (The first gather is gated at ~16.5 us by the Q7 mlp library
            # load, not by the idx DMA — warmup DMAs measurably don't help.)
            idx_sb = main.tile([P, SCHUNKS * IDX_COLS_SC], I16, tag="idx")
            nc.sync.dma_start(idx_sb[:], idxs_t[:])
            vaug13 = main.tile([N_DENSE, ROW], F32)
            nc.scalar.dma_start(vaug13[:], vaug_t[0:N_DENSE, :])
            dt_sb = main.tile([N_DENSE, BC], F32)
            nc.scalar.dma_start(dt_sb[:], dense_t[:])
            ident = main.tile([P, P], F32)
            nc.scalar.dma_start(ident[:], ident_t[:])

            # hoist num_idxs registers: one MOVE per distinct call size
            nf_sizes = sorted({nf for fg in (FGROUPS, FGROUPS_FIRST, FGROUPS_LAST)
                               for nf in fg})
            nidx_regs = {nf: nc.gpsimd.to_reg(nf * P) for nf in nf_sizes}

            res = main.tile([P, SCHUNKS], F32)

            # Warmup gather: zeroed idxs (no DMA dependency) gather row 0 into
            # a scratch tile as soon as the Q7 mlp library lands.  Warms the
            # ucode icache so the first real calls desc-gen at ~2.5 ns/row
            # instead of ~10, and primes the SDMA path before the stream.
            zidx = main.tile([P, P // 16], I16, tag="zidx")
            nc.gpsimd.memset(zidx[:], 0)
            gw = gtail.tile([P, 2, ROW], F32, tag="gt")
            nc.gpsimd.dma_gather(
                gw[:, :1, :], vaug_t[:], zidx[:], P, nidx_regs[1], ROW,
                single_packet=False, queue_num=0,
            )

            def fold_call(g, nf, gi):
                """DVE fold: call's nf gathered cols -> one column."""
                if nf == 1:
                    return g[:, 0, :]
                t1 = fold.tile([P, 3, ROW], F32, tag="t1")
                t2 = small.tile([P, ROW], F32, tag=f"t2_{gi % 2}")
                if nf >= 6:
                    nc.vector.tensor_add(t1[:], g[:, 0:3, :], g[:, 3:6, :])
                    nc.vector.tensor_add(t2[:], t1[:, 0, :], t1[:, 1, :])
                    nc.vector.tensor_add(t2[:], t2[:], t1[:, 2, :])
                    if nf == 7:
                        nc.vector.tensor_add(t2[:], t2[:], g[:, 6, :])
                elif nf in (4, 5):
                    nc.vector.tensor_add(t1[:, 0:2, :], g[:, 0:2, :], g[:, 2:4, :])
                    nc.vector.tensor_add(t2[:], t1[:, 0, :], t1[:, 1, :])
                    if nf == 5:
                        nc.vector.tensor_add(t2[:], t2[:], g[:, 4, :])
                else:  # nf == 2
                    nc.vector.tensor_add(t2[:], g[:, 0, :], g[:, 1, :])
                return t2

            def epilogue(c, fld):
                """FM identity epilogue for chunk c given complete fld."""
                blk = fld[:, 0:VCOLS]             # [128, 624] = field_f
                sq = fold.tile([P, VCOLS], F32, tag="sq")
                q = small.tile([P, 1], F32, tag="q")
                nc.scalar.activation(
                    sq[:], blk, mybir.ActivationFunctionType.Square,
                    accum_out=q[:],
                )
                # s_k for k=1..15 rides the gather in pad cols 625-639
                # (host packs sum_f v[a,f,k] there); only s_0 needs an
                # on-chip reduce, over 39 strided elements instead of 624.
                s0 = small.tile([P, 1], F32, tag="s0")
                blk_kf = blk.rearrange("p (f k) -> p k f", k=16)
                nc.vector.tensor_reduce(
                    out=s0[:], in_=blk_kf[:, 0:1, :], op=mybir.AluOpType.add,
                    axis=mybir.AxisListType.X,
                )
                s2 = small.tile([P, 15], F32, tag="s2")
                sn15 = small.tile([P, 1], F32, tag="sn15")
                nc.scalar.activation(
                    s2[:], fld[:, VCOLS + 1 : VCOLS + 16],
                    mybir.ActivationFunctionType.Square,
                    accum_out=sn15[:],
                )
                s0sq = small.tile([P, 1], F32, tag="s0sq")
                nc.vector.tensor_tensor(
                    out=s0sq[:], in0=s0[:], in1=s0[:],
                    op=mybir.AluOpType.mult,
                )
                # diff = (sn15 + s0^2) - q  ==  ||s||^2 - sum field_f^2
                diff = small.tile([P, 1], F32, tag="diff")
                nc.vector.scalar_tensor_tensor(
                    out=diff[:], in0=sn15[:], scalar=s0sq[:], in1=q[:],
                    op0=mybir.AluOpType.add, op1=mybir.AluOpType.subtract,
                )
                # out = 0.5*diff + (w-sum incl. w0 and dense first-order)
                nc.scalar.activation(
                    res[:, c : c + 1],
                    diff[:],
                    mybir.ActivationFunctionType.Identity,
                    bias=fld[:, VCOLS : VCOLS + 1],
                    scale=0.5,
                )
                nc.sync.dma_start(out_t[:, c : c + 1], res[:, c : c + 1])

            ps_c = {}      # chunk -> psum tile
            icol_c = {}    # chunk -> idx column offset consumed
            c3_cols = []   # chunk 3's DVE-chained columns (groups 3+)
            call_no = 1    # warmup gather above was call 0 on queue 0
            for c, gi in ISSUE:
                fgroups = CHUNK_GROUPS[c]
                ngroups = len(fgroups)
                nf = fgroups[gi]
                if gi == 0:
                    # dense part seeds this chunk's PSUM accumulation chain
                    ps = psum.tile([P, ROW], F32, tag="ps")
                    ps_c[c] = ps
                    icol_c[c] = 0
                    lhs_d = dt_sb[:, c * P : (c + 1) * P]
                    nc.tensor.matmul(out=ps[:, 0:512], lhsT=lhs_d,
                                     rhs=vaug13[:, 0:512], start=True, stop=False)
                    nc.tensor.matmul(out=ps[:, 512:ROW], lhsT=lhs_d,
                                     rhs=vaug13[:, 512:ROW], start=True, stop=False)
                ps = ps_c[c]
                icol = icol_c[c]

                n_idx = nf * P
                if nf <= 2:
                    g = gtail.tile([P, 2, ROW], F32, tag="gt")
                else:
                    g = gath.tile([P, 7, ROW], F32, tag="g")
                nc.gpsimd.dma_gather(
                    g[:, :nf, :],
                    vaug_t[:],
                    idx_sb[:, c * IDX_COLS_SC + icol
                           : c * IDX_COLS_SC + icol + n_idx // 16],
                    n_idx,
                    nidx_regs[nf],
                    ROW,
                    single_packet=False,
                    queue_num=call_no % N_QUEUES,
                )
                icol_c[c] = icol + n_idx // 16
                call_no += 1

                t2 = fold_call(g, nf, gi)

                if c < 3:
                    # PE accumulates all but the final group's col; final
                    # col joins via the DVE PSUM read (keeps PE off the
                    # chunk tail).
                    if gi < ngroups - 1:
                        acc_mm(ps, t2, start=False, stop=gi == ngroups - 2)
                    else:
                        fld = fold.tile([P, ROW], F32, tag="fld")
                        nc.vector.tensor_add(fld[:], t2[:], ps[:])
                        epilogue(c, fld)
                else:
                    # chunk 3: only the three big groups go through PE (the
                    # stop fires mid-stream); the tiny tail groups fold on
                    # DVE into fld in ARRIVAL order starting from the PSUM
                    # read, so after the final single-field call lands only
                    # one 640-col add + the epilogue remain.
                    if gi <= 2:
                        acc_mm(ps, t2, start=False, stop=gi == 2)
                    elif gi == 3:
                        fld3 = fold.tile([P, ROW], F32, tag="fld")
                        nc.vector.tensor_add(fld3[:], t2[:], ps[:])
                        c3_cols.append(fld3)
                    else:
                        fld3 = c3_cols[0]
                        nc.vector.tensor_add(fld3[:], fld3[:], t2[:])
                        if gi == ngroups - 1:
                            epilogue(3, fld3)

    nc.compile()
    return nc


def prep_inputs(dense_inputs, sparse_inputs, w0, w, v):
    """Host-side shard/pack: build per-core in_maps."""
    dense = np.asarray(dense_inputs, np.float32)
    sparse = np.asarray(sparse_inputs)
    w0 = np.asarray(w0, np.float32)
    w = np.asarray(w, np.float32)
    v = np.asarray(v, np.float32)

    vaug = np.zeros((FEATURE_NUM, ROW), np.float32)
    vaug[:, :VCOLS] = v.reshape(FEATURE_NUM, VCOLS)
    vaug[:, VCOLS] = w[:, 0]
    # fold w0 into sparse table 0 (each sample hits it exactly once)
    vaug[N_DENSE : N_DENSE + FEAT_PER_SPARSE, VCOLS] += w0[0]
    # pad cols 625-639 carry sum_f v[a,f,k] for k=1..15: the fold pipeline
    # then produces s_k for free and the epilogue only reduces s_0 on-chip
    vaug[:, VCOLS + 1 :] = v.sum(axis=1)[:, 1:16]

    offs = N_DENSE + FEAT_PER_SPARSE * np.arange(N_SPARSE, dtype=np.int64)
    gidx = (sparse.astype(np.int64) + offs[None, :]).astype(np.int16)  # [B, 26]

    in_maps = []
    for core in range(N_CORES):
        sl = slice(core * BC, (core + 1) * BC)
        dt = np.ascontiguousarray(dense[sl].T)          # [13, 512]
        idxc = gidx[sl]                                 # [512, 26]
        buf = np.zeros((P, SCHUNKS * IDX_COLS_SC), np.int16)
        off_c = 0
        for c in range(SCHUNKS):
            rows = idxc[c * P : (c + 1) * P]            # [128, 26]
            fbase = 0
            for nf in (FGROUPS_FIRST if c == 0 else
                       (FGROUPS_LAST if c == SCHUNKS - 1 else FGROUPS)):
                n = nf * P
                # call order: i = f_local*128 + p  ->  row idx[p, fbase+f]
                seg = np.ascontiguousarray(
                    rows[:, fbase : fbase + nf].T
                ).reshape(-1)                           # [nf*128]
                wrapped = seg.reshape(n // 16, 16).T    # [16, n/16]
                buf[:, off_c : off_c + n // 16] = np.tile(wrapped, (8, 1))
                fbase += nf
                off_c += n // 16
        in_maps.append({"vaug": vaug, "dense_t": dt, "idxs": buf,
                        "ident": np.eye(P, dtype=np.float32)})
    return in_maps


_NC_CACHE = None


def kernel(dense_inputs, sparse_inputs, w0, w, v):
    global _NC_CACHE
    if _NC_CACHE is None:
        _NC_CACHE = build_program()
    nc = _NC_CACHE
    in_maps = prep_inputs(dense_inputs, sparse_inputs, w0, w, v)
    res = run_bass_kernel_spmd(nc, in_maps, core_ids=list(range(N_CORES)))
    outs = []
    for r in res.results:
        o = r["out"]                                    # [128, 4]
        outs.append(np.ascontiguousarray(o.T).reshape(BC, 1))
    return np.concatenate(outs, axis=0).astype(np.float32)
